# Initial kernel scaffold
#
"""Trainium2 Bass kernel: GPT-style causal self-attention block.

Computes, for x[B=4, T=2048, C=1024], 16 heads x 64 dims:
    qkv = x @ w_attn + b_attn ; causal softmax attention ; y @ w_proj + b_proj

Sharding (8 cores): data-parallel over B (4) x tensor-parallel over head
groups (2 groups of 8 heads, Megatron style).  Each core:
  - computes Q^T/K^T (head-pair packed on partitions) and token-major V
    for its 8 heads from its batch's x,
  - runs causal attention per head: S^T[k,q] tiles -> exp on ScalarE
    (bounded scores; no max-subtraction needed) -> AV matmul with a
    [V | ones] stationary so the softmax denominators fall out of the
    same matmul -> normalize,
  - applies its row-slice of w_proj (row-parallel) producing a partial
    [T, C] output.  Host sums the two partials per batch and adds b_proj.
"""

import os
import numpy as np

B, T, C = 4, 2048, 1024
N_HEAD = 16
D = 64  # head dim
H_LOC = 8  # heads per core
N_CORES = 8

_cache = {}

# Set KERNEL_TRACE=1 to capture an NTFF profile; exec time lands in
# kernel.last_exec_ns.
last_exec_ns = None


def _build_program():
    from contextlib import ExitStack

    import concourse.bass as bass
    import concourse.mybir as mybir
    import concourse.tile as tile
    from concourse import bacc
    from concourse.masks import make_identity

    f32 = mybir.dt.float32
    bf16 = mybir.dt.bfloat16
    AF = mybir.ActivationFunctionType

    nc = bacc.Bacc("TRN2", target_bir_lowering=False, debug=False,
                   num_devices=N_CORES)

    x_d = nc.dram_tensor("x", [T, C], f32, kind="ExternalInput")
    wqkv_d = nc.dram_tensor("wqkv", [C, 1536], f32, kind="ExternalInput")
    bqkv_d = nc.dram_tensor("bqkv", [1536], f32, kind="ExternalInput")
    wp_d = nc.dram_tensor("wproj", [512, C], f32, kind="ExternalInput")
    out_d = nc.dram_tensor("out", [T, C], f32, kind="ExternalOutput")

    NTB = T // 128          # 16 token blocks
    NCB = C // 128          # 8 contraction blocks
    NMB = 8                 # q/k output blocks (pair-packed)

    with ExitStack() as ctx:
        tc = ctx.enter_context(tile.TileContext(nc))

        const = ctx.enter_context(tc.tile_pool(name="const", bufs=1))
        big = ctx.enter_context(tc.tile_pool(name="big", bufs=1))
        stream = ctx.enter_context(tc.tile_pool(name="stream", bufs=2))
        ptp = ctx.enter_context(tc.tile_pool(name="ptp", bufs=3))
        outp = ctx.enter_context(tc.tile_pool(name="outp", bufs=3))

        # ---- constants ----
        ident = const.tile([128, 128], f32)
        make_identity(nc, ident)
        # tri[k, q] = 1.0 where q >= k else 0  (valid-causal multiplicative
        # mask for the diagonal 128x128 block of an S^T tile)
        tri = const.tile([128, 128], bf16)
        nc.gpsimd.memset(tri, 1.0)
        nc.gpsimd.affine_select(
            out=tri, in_=tri, compare_op=mybir.AluOpType.is_ge,
            fill=0.0, base=0, pattern=[[1, 128]], channel_multiplier=-1,
        )
        ones1 = const.tile([1, 128], bf16)
        nc.gpsimd.memset(ones1, 1.0)

        # qk bias, one column per m-block: bqk_sb[p, mb] = bqkv[mb*128 + p]
        bqk_sb = const.tile([128, 8], f32)
        nc.sync.dma_start(bqk_sb, bqkv_d[0:1024].rearrange("(mb p) -> p mb", p=128))
        bv_f = const.tile([1, 512], f32)
        nc.sync.dma_start(bv_f, bqkv_d[None, 1024:1536])
        bv_sb = const.tile([1, 512], bf16)
        nc.vector.tensor_copy(bv_sb, bv_f)

        # ---- persistent tensors ----
        xT = big.tile([128, NCB, T], bf16)        # x^T, c on partitions
        wqk_sb = big.tile([128, NCB, 1024], bf16)
        wv_sb = big.tile([128, NCB, 512], bf16)
        wp_sb = big.tile([128, 4, 1024], bf16)
        qkT = big.tile([128, NMB, T], bf16)       # Q^T/K^T pair-packed
        v_sb = big.tile([128, H_LOC, NTB, 65], bf16)  # token-major V | ones
        yt = big.tile([128, 4, T], bf16)          # y^T pair-packed

        nc.gpsimd.memset(v_sb[:, :, :, 64:65], 1.0)

        # ---- weight loads + converts ----
        for cb in range(NCB):
            w_f = stream.tile([128, 1536], f32, name="w_f")
            nc.sync.dma_start(w_f, wqkv_d[cb * 128:(cb + 1) * 128, :])
            nc.vector.tensor_copy(wqk_sb[:, cb, :], w_f[:, 0:1024])
            nc.vector.tensor_copy(wv_sb[:, cb, :], w_f[:, 1024:1536])
        for p in range(4):
            w_f = stream.tile([128, 1536], f32, name="w_f")
            nc.sync.dma_start(w_f[:, 0:1024], wp_d[p * 128:(p + 1) * 128, :])
            nc.vector.tensor_copy(wp_sb[:, p, :], w_f[:, 0:1024])

        # ---- phase A: load x, transpose to xT (bf16) ----
        with tc.tile_pool(name="ps_a", bufs=2, space="PSUM") as ps_a:
            for tb in range(NTB):
                x_f = stream.tile([128, 1024], f32, name="x_f")
                nc.sync.dma_start(x_f, x_d[tb * 128:(tb + 1) * 128, :])
                xp = ps_a.tile([128, 1024], f32, name="xp")
                for cb in range(NCB):
                    nc.tensor.transpose(
                        xp[:, cb * 128:(cb + 1) * 128],
                        x_f[:, cb * 128:(cb + 1) * 128], ident)
                # one strided copy: [128, cb, 128t] -> xT[:, cb, tb*128:+128]
                nc.vector.tensor_copy(
                    xT[:, :, tb * 128:(tb + 1) * 128],
                    xp.rearrange("p (cb t) -> p cb t", cb=NCB))

        # ---- phase B: QKV projections ----
        with tc.tile_pool(name="ps_b", bufs=4, space="PSUM") as ps_b:
            # Q^T / K^T: stationary = w block, moving = x^T
            for mb in range(NMB):
                for ts in range(T // 512):
                    qp = ps_b.tile([128, 512], f32, name="qp")
                    for cb in range(NCB):
                        nc.tensor.matmul(
                            qp, wqk_sb[:, cb, mb * 128:(mb + 1) * 128],
                            xT[:, cb, ts * 512:(ts + 1) * 512],
                            start=(cb == 0), stop=(cb == NCB - 1))
                    # fused psum->sbuf copy + per-partition bias, on ScalarE
                    nc.scalar.activation(
                        qkT[:, mb, ts * 512:(ts + 1) * 512], qp,
                        AF.Identity, bias=bqk_sb[:, mb:mb + 1])
            # V token-major: stationary = x^T block, moving = wv
            for tb in range(NTB):
                vp = ps_b.tile([128, 512], f32, name="vp")
                for cb in range(NCB):
                    nc.tensor.matmul(
                        vp, xT[:, cb, tb * 128:(tb + 1) * 128],
                        wv_sb[:, cb, :], start=(cb == 0), stop=False)
                # bias via K=1 matmul: ones1^T @ bv  (adds bv to every row)
                nc.tensor.matmul(vp, ones1, bv_sb, start=False, stop=True)
                nc.scalar.activation(
                    v_sb[:, :, tb, 0:64],
                    vp.rearrange("p (h d) -> p h d", h=H_LOC),
                    AF.Identity, bias=0.0)

        # ---- phase C: attention per head, q in halves ----
        QH = 1024
        with tc.tile_pool(name="ps_s", bufs=2, space="PSUM") as ps_s, \
             tc.tile_pool(name="ps_y", bufs=2, space="PSUM") as ps_y:
            for h in range(H_LOC):
                pr = h // 2           # pair index
                po = (h % 2) * 64     # partition offset within pair
                q_mb, k_mb = 2 * pr, 2 * pr + 1
                qT = qkT[po:po + 64, q_mb, :]
                kT = qkT[po:po + 64, k_mb, :]
                for qh in range(T // QH):
                    q0 = qh * QH
                    nkb = (q0 + QH) // 128
                    y_ps = ps_y.tile([128, QH], f32, name="y_ps")
                    for kb in range(nkb):
                        qlo = max(q0, kb * 128)
                        qlen = q0 + QH - qlo
                        s_ps = ps_s.tile([128, QH], f32, name="s_ps")
                        # S^T tiles (<=512-wide matmuls, one psum bank each)
                        for s0 in range(0, qlen, 512):
                            sl = min(512, qlen - s0)
                            nc.tensor.matmul(
                                s_ps[:, s0:s0 + sl],
                                kT[:, kb * 128:(kb + 1) * 128],
                                qT[:, qlo + s0:qlo + s0 + sl],
                                start=True, stop=True)
                        pt = ptp.tile([128, QH], bf16, name="pt")
                        nc.scalar.activation(pt[:, 0:qlen], s_ps[:, 0:qlen],
                                             AF.Exp, scale=0.125)
                        if kb * 128 >= q0:
                            # diagonal block: zero the strictly-upper part
                            nc.gpsimd.tensor_mul(pt[:, 0:128], pt[:, 0:128], tri)
                        # AV (+ sums in row 64): segments aligned to psum banks
                        off = qlo - q0
                        s0 = off
                        while s0 < QH:
                            s1 = min(QH, (s0 // 512 + 1) * 512)
                            nc.tensor.matmul(
                                y_ps[0:65, s0:s1], v_sb[:, h, kb, :],
                                pt[:, s0 - off:s1 - off],
                                start=(kb == 0), stop=(kb == nkb - 1),
                                skip_group_check=True)
                            s0 = s1
                    # normalize: recip of sums row, broadcast, scale
                    recip = stream.tile([1, QH], f32, name="recip")
                    nc.vector.reciprocal_approx_fast(recip, y_ps[64:65, :])
                    bc = stream.tile([64, QH], f32, name="bc")
                    nc.gpsimd.partition_broadcast(bc, recip)
                    nc.vector.tensor_mul(
                        yt[po:po + 64, pr, q0:q0 + QH], y_ps[0:64, :], bc)

        # ---- phase D: output projection (row-parallel partial) ----
        with tc.tile_pool(name="ps_p", bufs=4, space="PSUM") as ps_p:
            for tb in range(NTB):
                for ns in range(2):
                    pp = ps_p.tile([128, 512], f32, name="pp")
                    for p in range(4):
                        nc.tensor.matmul(
                            pp, yt[:, p, tb * 128:(tb + 1) * 128],
                            wp_sb[:, p, ns * 512:(ns + 1) * 512],
                            start=(p == 0), stop=(p == 3))
                    o_sb = outp.tile([128, 512], f32, name="o_sb")
                    nc.vector.tensor_copy(o_sb, pp)
                    nc.sync.dma_start(
                        out_d[tb * 128:(tb + 1) * 128, ns * 512:(ns + 1) * 512],
                        o_sb)

    nc.compile()
    return nc


def _shard_inputs(x, w_attn, b_attn, w_proj):
    """Build per-core input maps (pair-packed q/k layouts; see module doc)."""
    wq = w_attn[:, 0:C].reshape(C, N_HEAD, D)
    wk = w_attn[:, C:2 * C].reshape(C, N_HEAD, D)
    wv = w_attn[:, 2 * C:3 * C].reshape(C, N_HEAD, D)
    bq = b_attn[0:C].reshape(N_HEAD, D)
    bk = b_attn[C:2 * C].reshape(N_HEAD, D)
    bv = b_attn[2 * C:3 * C].reshape(N_HEAD, D)

    in_maps = []
    for core in range(N_CORES):
        b, g = core // 2, core % 2
        h0 = g * H_LOC
        qk_blocks, bqk_parts = [], []
        for p in range(4):
            hA, hB = h0 + 2 * p, h0 + 2 * p + 1
            qk_blocks.append(np.concatenate([wq[:, hA], wq[:, hB]], axis=1))
            qk_blocks.append(np.concatenate([wk[:, hA], wk[:, hB]], axis=1))
            bqk_parts.append(np.concatenate([bq[hA], bq[hB]]))
            bqk_parts.append(np.concatenate([bk[hA], bk[hB]]))
        wqkv = np.concatenate(
            qk_blocks + [wv[:, h0:h0 + H_LOC].reshape(C, H_LOC * D)], axis=1)
        bqkv = np.concatenate(
            bqk_parts + [bv[h0:h0 + H_LOC].reshape(H_LOC * D)])
        wproj = w_proj.reshape(N_HEAD, D, C)[h0:h0 + H_LOC].reshape(H_LOC * D, C)
        in_maps.append({
            "x": np.ascontiguousarray(x[b], dtype=np.float32),
            "wqkv": np.ascontiguousarray(wqkv, dtype=np.float32),
            "bqkv": np.ascontiguousarray(bqkv, dtype=np.float32),
            "wproj": np.ascontiguousarray(wproj, dtype=np.float32),
        })
    return in_maps


def kernel(x, w_attn, b_attn, w_proj, b_proj):
    global last_exec_ns
    from concourse.bass_utils import run_bass_kernel_spmd

    x = np.asarray(x, dtype=np.float32)
    w_attn = np.asarray(w_attn, dtype=np.float32)
    b_attn = np.asarray(b_attn, dtype=np.float32)
    w_proj = np.asarray(w_proj, dtype=np.float32)
    b_proj = np.asarray(b_proj, dtype=np.float32)

    if "nc" not in _cache:
        _cache["nc"] = _build_program()
    nc = _cache["nc"]

    in_maps = _shard_inputs(x, w_attn, b_attn, w_proj)
    trace = os.environ.get("KERNEL_TRACE", "0") == "1"
    res = run_bass_kernel_spmd(nc, in_maps, core_ids=list(range(N_CORES)),
                               trace=trace)
    last_exec_ns = res.exec_time_ns

    out = np.empty((B, T, C), dtype=np.float32)
    for b in range(B):
        out[b] = (res.results[2 * b]["out"] + res.results[2 * b + 1]["out"]
                  + b_proj[None, :])
    return out


# revision 5
# speedup vs baseline: 1.0029x; 1.0029x over previous
"""Trainium2 Bass kernel: GPT-style causal self-attention block.

Computes, for x[B=4, T=2048, C=1024], 16 heads x 64 dims:
    qkv = x @ w_attn + b_attn ; causal softmax attention ; y @ w_proj + b_proj

Sharding (8 cores): data-parallel over B (4) x tensor-parallel over head
groups (2 groups of 8 heads, Megatron style).  Each core:
  - computes Q^T/K^T (head-pair packed on partitions) and token-major V
    for its 8 heads from its batch's x,
  - runs causal attention per head: S^T[k,q] tiles -> exp on ScalarE
    (bounded scores; no max-subtraction needed) -> AV matmul with a
    [V | ones] stationary so the softmax denominators fall out of the
    same matmul -> normalize,
  - applies its row-slice of w_proj (row-parallel) producing a partial
    [T, C] output.  Host sums the two partials per batch and adds b_proj.
"""

import os
import numpy as np

B, T, C = 4, 2048, 1024
N_HEAD = 16
D = 64  # head dim
H_LOC = 8  # heads per core
N_CORES = 8

_cache = {}

# Set KERNEL_TRACE=1 to capture an NTFF profile; exec time lands in
# kernel.last_exec_ns.
last_exec_ns = None


def _build_program(reps=1):
    from contextlib import ExitStack

    import concourse.bass as bass
    import concourse.mybir as mybir
    import concourse.tile as tile
    from concourse import bacc
    from concourse.masks import make_identity

    f32 = mybir.dt.float32
    bf16 = mybir.dt.bfloat16
    AF = mybir.ActivationFunctionType

    nc = bacc.Bacc("TRN2", target_bir_lowering=False, debug=False,
                   num_devices=N_CORES)

    x_d = nc.dram_tensor("x", [T, C], f32, kind="ExternalInput")
    wqkv_d = nc.dram_tensor("wqkv", [C, 1536], f32, kind="ExternalInput")
    bqkv_d = nc.dram_tensor("bqkv", [1536], f32, kind="ExternalInput")
    wp_d = nc.dram_tensor("wproj", [512, C], f32, kind="ExternalInput")
    out_d = nc.dram_tensor("out", [T, C], f32, kind="ExternalOutput")

    NTB = T // 128          # 16 token blocks
    NCB = C // 128          # 8 contraction blocks
    NMB = 8                 # q/k output blocks (pair-packed)

    with ExitStack() as ctx:
        tc = ctx.enter_context(tile.TileContext(nc))

        const = ctx.enter_context(tc.tile_pool(name="const", bufs=1))
        big = ctx.enter_context(tc.tile_pool(name="big", bufs=1))
        stream = ctx.enter_context(tc.tile_pool(name="stream", bufs=2))
        ptp = ctx.enter_context(tc.tile_pool(name="ptp", bufs=3))
        outp = ctx.enter_context(tc.tile_pool(name="outp", bufs=3))

        # ---- constants ----
        ident = const.tile([128, 128], f32)
        make_identity(nc, ident)
        # tri[k, q] = 1.0 where q >= k else 0  (valid-causal multiplicative
        # mask for the diagonal 128x128 block of an S^T tile)
        tri = const.tile([128, 128], bf16)
        nc.gpsimd.memset(tri, 1.0)
        nc.gpsimd.affine_select(
            out=tri, in_=tri, compare_op=mybir.AluOpType.is_ge,
            fill=0.0, base=0, pattern=[[1, 128]], channel_multiplier=-1,
        )
        ones1 = const.tile([1, 128], bf16)
        nc.gpsimd.memset(ones1, 1.0)

        # qk bias, one column per m-block: bqk_sb[p, mb] = bqkv[mb*128 + p]
        bqk_sb = const.tile([128, 8], f32)
        nc.sync.dma_start(bqk_sb, bqkv_d[0:1024].rearrange("(mb p) -> p mb", p=128))
        bv_f = const.tile([1, 512], f32)
        nc.sync.dma_start(bv_f, bqkv_d[None, 1024:1536])
        bv_sb = const.tile([1, 512], bf16)
        nc.vector.tensor_copy(bv_sb, bv_f)

        for _rep in range(reps):
            _emit_body(nc, tc, mybir, AF, f32, bf16, make_identity,
                       const, big, stream, ptp, outp,
                       x_d, wqkv_d, bqkv_d, wp_d, out_d,
                       ident, tri, ones1, bqk_sb, bv_sb, NTB, NCB, NMB)

    nc.compile()
    return nc


def _emit_body(nc, tc, mybir, AF, f32, bf16, make_identity,
               const, big, stream, ptp, outp,
               x_d, wqkv_d, bqkv_d, wp_d, out_d,
               ident, tri, ones1, bqk_sb, bv_sb, NTB, NCB, NMB):
    if True:  # keep original indentation below
        # ---- persistent tensors ----
        xT = big.tile([128, NCB, T], bf16, name="xT")  # x^T, c on partitions
        wqk_sb = big.tile([128, NCB, 1024], bf16, name="wqk_sb")
        wv_sb = big.tile([128, NCB, 512], bf16, name="wv_sb")
        wp_sb = big.tile([128, 4, 1024], bf16, name="wp_sb")
        qkT = big.tile([128, NMB, T], bf16, name="qkT")  # Q^T/K^T pair-packed
        v_sb = big.tile([128, H_LOC, NTB, 65], bf16, name="v_sb")
        yt = big.tile([128, 4, T], bf16, name="yt")      # y^T pair-packed

        nc.gpsimd.memset(v_sb[:, :, :, 64:65], 1.0)

        # ---- weight loads + converts ----
        for cb in range(NCB):
            w_f = stream.tile([128, 1536], f32, name="w_f")
            nc.sync.dma_start(w_f, wqkv_d[cb * 128:(cb + 1) * 128, :])
            nc.vector.tensor_copy(wqk_sb[:, cb, :], w_f[:, 0:1024])
            nc.vector.tensor_copy(wv_sb[:, cb, :], w_f[:, 1024:1536])
        for p in range(4):
            w_f = stream.tile([128, 1536], f32, name="w_f")
            nc.sync.dma_start(w_f[:, 0:1024], wp_d[p * 128:(p + 1) * 128, :])
            nc.vector.tensor_copy(wp_sb[:, p, :], w_f[:, 0:1024])

        # ---- phase A: load x, transpose to xT (bf16) ----
        with tc.tile_pool(name="ps_a", bufs=2, space="PSUM") as ps_a:
            for tb in range(NTB):
                x_f = stream.tile([128, 1024], f32, name="x_f")
                nc.sync.dma_start(x_f, x_d[tb * 128:(tb + 1) * 128, :])
                xp = ps_a.tile([128, 1024], f32, name="xp")
                for cb in range(NCB):
                    nc.tensor.transpose(
                        xp[:, cb * 128:(cb + 1) * 128],
                        x_f[:, cb * 128:(cb + 1) * 128], ident)
                # one strided copy: [128, cb, 128t] -> xT[:, cb, tb*128:+128]
                nc.vector.tensor_copy(
                    xT[:, :, tb * 128:(tb + 1) * 128],
                    xp.rearrange("p (cb t) -> p cb t", cb=NCB))

        # ---- phase B: QKV projections ----
        with tc.tile_pool(name="ps_b", bufs=4, space="PSUM") as ps_b:
            # Q^T / K^T: stationary = w block, moving = x^T
            for mb in range(NMB):
                for ts in range(T // 512):
                    qp = ps_b.tile([128, 512], f32, name="qp")
                    for cb in range(NCB):
                        nc.tensor.matmul(
                            qp, wqk_sb[:, cb, mb * 128:(mb + 1) * 128],
                            xT[:, cb, ts * 512:(ts + 1) * 512],
                            start=(cb == 0), stop=(cb == NCB - 1))
                    # fused psum->sbuf copy + per-partition bias, on ScalarE
                    nc.scalar.activation(
                        qkT[:, mb, ts * 512:(ts + 1) * 512], qp,
                        AF.Identity, bias=bqk_sb[:, mb:mb + 1])
            # V token-major: stationary = x^T block, moving = wv
            for tb in range(NTB):
                vp = ps_b.tile([128, 512], f32, name="vp")
                for cb in range(NCB):
                    nc.tensor.matmul(
                        vp, xT[:, cb, tb * 128:(tb + 1) * 128],
                        wv_sb[:, cb, :], start=(cb == 0), stop=False)
                # bias via K=1 matmul: ones1^T @ bv  (adds bv to every row)
                nc.tensor.matmul(vp, ones1, bv_sb, start=False, stop=True)
                nc.scalar.activation(
                    v_sb[:, :, tb, 0:64],
                    vp.rearrange("p (h d) -> p h d", h=H_LOC),
                    AF.Identity, bias=0.0)

        # ---- phase C: attention per head, q in halves ----
        QH = 1024
        with tc.tile_pool(name="ps_s", bufs=2, space="PSUM") as ps_s, \
             tc.tile_pool(name="ps_y", bufs=2, space="PSUM") as ps_y:
            for h in range(H_LOC):
                pr = h // 2           # pair index
                po = (h % 2) * 64     # partition offset within pair
                q_mb, k_mb = 2 * pr, 2 * pr + 1
                qT = qkT[po:po + 64, q_mb, :]
                kT = qkT[po:po + 64, k_mb, :]
                for qh in range(T // QH):
                    q0 = qh * QH
                    nkb = (q0 + QH) // 128
                    y_ps = ps_y.tile([128, QH], f32, name="y_ps")
                    for kb in range(nkb):
                        qlo = max(q0, kb * 128)
                        qlen = q0 + QH - qlo
                        s_ps = ps_s.tile([128, QH], f32, name="s_ps")
                        # S^T tiles (<=512-wide matmuls, one psum bank each)
                        for s0 in range(0, qlen, 512):
                            sl = min(512, qlen - s0)
                            nc.tensor.matmul(
                                s_ps[:, s0:s0 + sl],
                                kT[:, kb * 128:(kb + 1) * 128],
                                qT[:, qlo + s0:qlo + s0 + sl],
                                start=True, stop=True)
                        pt = ptp.tile([128, QH], bf16, name="pt")
                        nc.scalar.activation(pt[:, 0:qlen], s_ps[:, 0:qlen],
                                             AF.Exp, scale=0.125)
                        if kb * 128 >= q0:
                            # diagonal block: zero the strictly-upper part
                            nc.gpsimd.tensor_mul(pt[:, 0:128], pt[:, 0:128], tri)
                        # AV (+ sums in row 64): segments aligned to psum banks
                        off = qlo - q0
                        s0 = off
                        while s0 < QH:
                            s1 = min(QH, (s0 // 512 + 1) * 512)
                            nc.tensor.matmul(
                                y_ps[0:65, s0:s1], v_sb[:, h, kb, :],
                                pt[:, s0 - off:s1 - off],
                                start=(kb == 0), stop=(kb == nkb - 1),
                                skip_group_check=True)
                            s0 = s1
                    # normalize: recip of sums row, broadcast, scale
                    # (custom-DVE recip can't read PSUM on HW; stage via SBUF)
                    sums_sb = stream.tile([1, QH], f32, name="sums_sb")
                    nc.vector.tensor_copy(sums_sb, y_ps[64:65, :])
                    recip = stream.tile([1, QH], f32, name="recip")
                    nc.vector.reciprocal_approx_fast(recip, sums_sb)
                    bc = stream.tile([64, QH], f32, name="bc")
                    nc.gpsimd.partition_broadcast(bc, recip)
                    nc.vector.tensor_mul(
                        yt[po:po + 64, pr, q0:q0 + QH], y_ps[0:64, :], bc)

        # ---- phase D: output projection (row-parallel partial) ----
        with tc.tile_pool(name="ps_p", bufs=4, space="PSUM") as ps_p:
            for tb in range(NTB):
                for ns in range(2):
                    pp = ps_p.tile([128, 512], f32, name="pp")
                    for p in range(4):
                        nc.tensor.matmul(
                            pp, yt[:, p, tb * 128:(tb + 1) * 128],
                            wp_sb[:, p, ns * 512:(ns + 1) * 512],
                            start=(p == 0), stop=(p == 3))
                    o_sb = outp.tile([128, 512], f32, name="o_sb")
                    nc.vector.tensor_copy(o_sb, pp)
                    nc.sync.dma_start(
                        out_d[tb * 128:(tb + 1) * 128, ns * 512:(ns + 1) * 512],
                        o_sb)


def _shard_inputs(x, w_attn, b_attn, w_proj):
    """Build per-core input maps (pair-packed q/k layouts; see module doc)."""
    wq = w_attn[:, 0:C].reshape(C, N_HEAD, D)
    wk = w_attn[:, C:2 * C].reshape(C, N_HEAD, D)
    wv = w_attn[:, 2 * C:3 * C].reshape(C, N_HEAD, D)
    bq = b_attn[0:C].reshape(N_HEAD, D)
    bk = b_attn[C:2 * C].reshape(N_HEAD, D)
    bv = b_attn[2 * C:3 * C].reshape(N_HEAD, D)

    in_maps = []
    for core in range(N_CORES):
        b, g = core // 2, core % 2
        h0 = g * H_LOC
        qk_blocks, bqk_parts = [], []
        for p in range(4):
            hA, hB = h0 + 2 * p, h0 + 2 * p + 1
            qk_blocks.append(np.concatenate([wq[:, hA], wq[:, hB]], axis=1))
            qk_blocks.append(np.concatenate([wk[:, hA], wk[:, hB]], axis=1))
            bqk_parts.append(np.concatenate([bq[hA], bq[hB]]))
            bqk_parts.append(np.concatenate([bk[hA], bk[hB]]))
        wqkv = np.concatenate(
            qk_blocks + [wv[:, h0:h0 + H_LOC].reshape(C, H_LOC * D)], axis=1)
        bqkv = np.concatenate(
            bqk_parts + [bv[h0:h0 + H_LOC].reshape(H_LOC * D)])
        wproj = w_proj.reshape(N_HEAD, D, C)[h0:h0 + H_LOC].reshape(H_LOC * D, C)
        in_maps.append({
            "x": np.ascontiguousarray(x[b], dtype=np.float32),
            "wqkv": np.ascontiguousarray(wqkv, dtype=np.float32),
            "bqkv": np.ascontiguousarray(bqkv, dtype=np.float32),
            "wproj": np.ascontiguousarray(wproj, dtype=np.float32),
        })
    return in_maps


def kernel(x, w_attn, b_attn, w_proj, b_proj):
    global last_exec_ns
    from concourse.bass_utils import run_bass_kernel_spmd

    x = np.asarray(x, dtype=np.float32)
    w_attn = np.asarray(w_attn, dtype=np.float32)
    b_attn = np.asarray(b_attn, dtype=np.float32)
    w_proj = np.asarray(w_proj, dtype=np.float32)
    b_proj = np.asarray(b_proj, dtype=np.float32)

    if "nc" not in _cache:
        _cache["nc"] = _build_program()
    nc = _cache["nc"]

    in_maps = _shard_inputs(x, w_attn, b_attn, w_proj)
    trace = os.environ.get("KERNEL_TRACE", "0") == "1"
    if trace:
        try:
            import antenv.axon_hooks  # noqa: F401
        except ImportError:
            trace = False
    res = run_bass_kernel_spmd(nc, in_maps, core_ids=list(range(N_CORES)),
                               trace=trace)
    last_exec_ns = res.exec_time_ns

    out = np.empty((B, T, C), dtype=np.float32)
    for b in range(B):
        out[b] = (res.results[2 * b]["out"] + res.results[2 * b + 1]["out"]
                  + b_proj[None, :])
    return out
